# revision 5
# baseline (speedup 1.0000x reference)
# Multi-head attention (B=4, L=2048, D=512, H=8, dh=64) on 8 trn2 cores.
# Sharding: core c -> batch b = c//2, head-group hg = c%2 (4 heads, 256 out
# channels). Host permutes q/wq feature columns so the residual slice is
# always columns 0:256 (SPMD-uniform).
#
# Per-core plan:
#   - q/k/w loaded natural fp32, cast to bf16 on Pool/Act, transposed via
#     DMA XBAR (dma_start_transpose) -> qT/kT/wT (no PE transposes).
#   - QT/KT projections (bf16) emit a head-shuffled layout: PSUM partition
#     h*32+k' for sub-row i, copy-cast to fp8e4m3 -> QT8/KT8 [128, 2*2048].
#   - S^T = K_h Q_h^T as fp8 DoubleRow matmuls (contraction 64 = 32x2),
#     256 q-columns per 128-key block at 0.5 cyc/col.
#   - exp: Act does exact exp->bf16; DVE does Schraudolph int16 bitcast bf16
#     (softmax ratio cancels the common-mode scale).
#   - PV natural orientation: lhsT = P^T chunk [128k,128q], rhs = V+ones
#     [128,65] -> PSUM [128q, 65] accumulated over 16 key blocks; col 64 is
#     the softmax denominator. Normalize+residual fused on DVE.
import sys

import numpy as np

sys.path.insert(0, "/opt/trn_rl_repo")

L = 2048
D = 512
NH = 4          # heads per core
DH = 64
DHG = NH * DH   # 256 output channels per core
NLT = L // 128  # 16 row tiles
NCI = D // 128  # 4 feature chunks
QB = 512        # q block
NQB = L // QB   # 4
INV_SCALE = 1.0 / float(np.sqrt(D))
# Schraudolph exp in bf16-int space: int16 = round(x*128/ln2 + 16255.4)
SCH_C1 = float(128.0 / np.log(2.0) * INV_SCALE)
SCH_C2 = float(127.0 * 128.0 - 0.6)

_cache = {}


def _build():
    import concourse.bacc as bacc
    import concourse.mybir as mybir
    import concourse.tile as tile

    f32 = mybir.dt.float32
    bf16 = mybir.dt.bfloat16
    i16 = mybir.dt.int16
    fp8 = mybir.dt.float8e4
    EXP = mybir.ActivationFunctionType.Exp
    CPY = mybir.ActivationFunctionType.Copy
    MUL = mybir.AluOpType.mult
    ADD = mybir.AluOpType.add
    DR = mybir.MatmulPerfMode.DoubleRow

    nc = bacc.Bacc("TRN2", target_bir_lowering=False, debug=False, num_devices=8)
    q_d = nc.dram_tensor("q", [L, D], f32, kind="ExternalInput").ap()
    k_d = nc.dram_tensor("k", [L, D], f32, kind="ExternalInput").ap()
    wq_d = nc.dram_tensor("wq", [DHG, D], f32, kind="ExternalInput").ap()
    wk_d = nc.dram_tensor("wk", [DHG, D], f32, kind="ExternalInput").ap()
    wv_d = nc.dram_tensor("wv", [DHG, D], f32, kind="ExternalInput").ap()
    o_d = nc.dram_tensor("o", [L, DHG], f32, kind="ExternalOutput").ap()

    with tile.TileContext(nc) as tc:
        with (
            tc.tile_pool(name="static", bufs=1) as st_pool,
            tc.tile_pool(name="bfc", bufs=4) as bf_pool,
            tc.tile_pool(name="ppool", bufs=5) as p_pool,
            tc.tile_pool(name="outsb", bufs=5) as out_pool,
            tc.tile_pool(name="recip", bufs=5) as r_pool,
            tc.tile_pool(name="ps_pv", bufs=4, space="PSUM") as ps_pv,
        ):
            # ---- static tiles
            qnat = [st_pool.tile([128, 512], f32, name=f"qn{lt}") for lt in range(NLT)]
            knat = [st_pool.tile([128, 512], f32, name=f"kn{lt}") for lt in range(NLT)]
            wnat = [st_pool.tile([128, 512], f32, name=f"wn{j}") for j in range(6)]
            qT = [st_pool.tile([128, L], bf16, name=f"qT{c}") for c in range(NCI)]
            kT = [st_pool.tile([128, L], bf16, name=f"kT{c}") for c in range(NCI)]
            # wT[x][c]: [128 d, 256 o] for w x in (q,k,v), d-chunk c
            wT = [
                [st_pool.tile([128, DHG], bf16, name=f"wT{x}{c}") for c in range(NCI)]
                for x in range(3)
            ]
            QT8 = st_pool.tile([128, 2 * L], fp8, name="QT8")
            KT8 = st_pool.tile([128, 2 * L], fp8, name="KT8")
            V_all = [
                st_pool.tile([128, NH * (DH + 1)], bf16, name=f"V{kt}")
                for kt in range(NLT)
            ]

            def vones(kt):
                view = V_all[kt].rearrange("p (h x) -> p h x", h=NH)[:, :, DH : DH + 1]
                nc.gpsimd.memset(view, 1.0)

            # ---- phase A: loads
            for j in range(2):
                nc.sync.dma_start(out=wnat[0 + j], in_=wq_d[j * 128 : (j + 1) * 128, :])
                nc.sync.dma_start(out=wnat[2 + j], in_=wk_d[j * 128 : (j + 1) * 128, :])
                nc.sync.dma_start(out=wnat[4 + j], in_=wv_d[j * 128 : (j + 1) * 128, :])
            for lt in range(NLT):
                nc.sync.dma_start(out=qnat[lt], in_=q_d[lt * 128 : (lt + 1) * 128, :])
            for lt in range(NLT):
                nc.sync.dma_start(out=knat[lt], in_=k_d[lt * 128 : (lt + 1) * 128, :])

            # ---- casts + dma transposes
            # w: cast on Pool, transpose on SP
            for x in range(3):
                for oc in range(2):
                    wb = bf_pool.tile([128, 512], bf16, tag="wb", name="wb")
                    nc.gpsimd.tensor_copy(wb, wnat[x * 2 + oc])
                    for c in range(NCI):
                        nc.sync.dma_start_transpose(
                            wT[x][c][:, oc * 128 : (oc + 1) * 128],
                            wb[:, c * 128 : (c + 1) * 128],
                        )
            for kt in range(NLT):
                vones(kt)
            # q: cast on Act
            for lt in range(NLT):
                qb_ = bf_pool.tile([128, 512], bf16, tag="qb", name="qb")
                nc.scalar.activation(qb_, qnat[lt], CPY)
                for c in range(NCI):
                    nc.sync.dma_start_transpose(
                        qT[c][:, lt * 128 : (lt + 1) * 128],
                        qb_[:, c * 128 : (c + 1) * 128],
                    )
            # k: cast on Pool
            for lt in range(NLT):
                kb_ = bf_pool.tile([128, 512], bf16, tag="kb", name="kb")
                nc.gpsimd.tensor_copy(kb_, knat[lt])
                for c in range(NCI):
                    nc.sync.dma_start_transpose(
                        kT[c][:, lt * 128 : (lt + 1) * 128],
                        kb_[:, c * 128 : (c + 1) * 128],
                    )

            def perm_lhsT(x, ci, i):
                # wq/wk rows are host-permuted to (i, h, k') order, so sub-row
                # i's 128 weight columns are contiguous
                return wT[x][ci][:, i * 128 : (i + 1) * 128]

            def proj_qk(x, tT, dst8, lb, i, copy_eng):
                ps = ps_k.tile([128, 512], f32, tag="s1", name="pjps")
                for ci in range(NCI):
                    nc.tensor.matmul(
                        ps,
                        lhsT=perm_lhsT(x, ci, i),
                        rhs=tT[ci][:, lb * QB : (lb + 1) * QB],
                        start=(ci == 0),
                        stop=(ci == NCI - 1),
                    )
                dst = dst8[:, i * L + lb * QB : i * L + (lb + 1) * QB]
                if copy_eng == "act":
                    nc.scalar.activation(dst, ps, CPY)
                else:
                    nc.vector.tensor_copy(dst, ps)

            def v_proj(kt, copy_eng):
                ps = ps_k.tile([128, 512], f32, tag="s1", name="vps")
                for ci in range(NCI):
                    nc.tensor.matmul(
                        ps[:, 0:DHG],
                        lhsT=kT[ci][:, kt * 128 : (kt + 1) * 128],
                        rhs=wT[2][ci],
                        start=(ci == 0),
                        stop=(ci == NCI - 1),
                    )
                dst = V_all[kt].rearrange("p (h x) -> p h x", h=NH)[:, :, 0:DH]
                src = ps[:, 0:DHG].rearrange("p (h x) -> p h x", h=NH)
                if copy_eng == "act":
                    nc.scalar.activation(dst, src, CPY)
                else:
                    nc.vector.tensor_copy(dst, src)

            def dr_views(t8, h):
                return t8[32 * h : 32 * (h + 1), :].rearrange(
                    "p (i l) -> p i l", i=2
                )

            def s_mm(h, kt, qb, out_ap):
                # S^T [128 keys, 512 q] fp8 DoubleRow
                nc.tensor.matmul(
                    out_ap,
                    lhsT=dr_views(KT8, h)[:, :, kt * 128 : (kt + 1) * 128],
                    rhs=dr_views(QT8, h)[:, :, qb * QB : (qb + 1) * QB],
                    start=True,
                    stop=True,
                    perf_mode=DR,
                    tile_position=(32 * h, 0),
                )

            def exp_to(p_dst, s_src, eng):
                if eng == "act":
                    nc.scalar.activation(p_dst, s_src, EXP, scale=INV_SCALE)
                else:
                    nc.vector.tensor_scalar(
                        p_dst.bitcast(i16), s_src, SCH_C1, SCH_C2, MUL, ADD
                    )

            def pv_mm(pv_t, h, kt, p_ap, qsb, first, last):
                nc.tensor.matmul(
                    pv_t[:, qsb * (DH + 1) : (qsb + 1) * (DH + 1)],
                    lhsT=p_ap[:, qsb * 128 : (qsb + 1) * 128],
                    rhs=V_all[kt][:, h * (DH + 1) : (h + 1) * (DH + 1)],
                    start=first,
                    stop=last,
                    skip_group_check=True,
                )

            def finish_unit(h, qb, pv_t, out_t):
                # reciprocal of the 4 row-sum columns, then normalize+residual
                rc = r_pool.tile([128, 4], f32, tag="rc", name="rc")
                sums = pv_t[:, 0 : 4 * (DH + 1)].rearrange(
                    "p (q x) -> p q x", x=DH + 1
                )[:, :, DH]
                nc.vector.reciprocal(rc, sums)
                for qsb in range(4):
                    nc.vector.scalar_tensor_tensor(
                        out=out_t[qsb][:, h * DH : (h + 1) * DH],
                        in0=pv_t[:, qsb * (DH + 1) : qsb * (DH + 1) + DH],
                        scalar=rc[:, qsb : qsb + 1],
                        in1=qnat[qb * 4 + qsb][:, h * DH : (h + 1) * DH],
                        op0=MUL,
                        op1=ADD,
                    )

            def out_tiles():
                return [
                    out_pool.tile([128, DHG], f32, tag="ot", name="ot") for _ in range(4)
                ]

            def dma_out(qb, out_t):
                for qsb in range(4):
                    lt = qb * 4 + qsb
                    nc.sync.dma_start(
                        out=o_d[lt * 128 : (lt + 1) * 128, :], in_=out_t[qsb]
                    )

            # ---- scope 1: projections + K production + qb0 attention
            with tc.tile_pool(name="ps_k", bufs=4, space="PSUM") as ps_k:
                # Q projections -> QT8
                for i in range(2):
                    for lb in range(NQB):
                        proj_qk(0, qT, QT8, lb, i, "dve")
                # K production interleaved with qb0 attention
                pv0 = [
                    ps_pv.tile([128, 512], f32, tag="pv", name=f"pv0{h}")
                    for h in range(NH)
                ]
                out0 = out_tiles()
                expi = 0
                for lb in range(NQB):
                    for i in range(2):
                        proj_qk(1, kT, KT8, lb, i, "dve")
                    for kt in range(lb * 4, lb * 4 + 4):
                        v_proj(kt, "act" if kt % 2 == 0 else "dve")
                    # qb0 attention for this lb's key blocks
                    for kt in range(lb * 4, lb * 4 + 4):
                        for h in range(NH):
                            s_t = ps_k.tile([128, 512], f32, tag="s1", name="s0")
                            s_mm(h, kt, 0, s_t)
                            p_t = p_pool.tile([128, 512], bf16, tag="p0", name="p0")
                            exp_to(p_t, s_t, "act" if expi % 2 == 0 else "dve")
                            expi += 1
                            for qsb in range(4):
                                pv_mm(
                                    pv0[h], h, kt, p_t, qsb,
                                    first=(kt == 0 and qsb == 0),
                                    last=(kt == NLT - 1 and qsb == 3),
                                )
                for h in range(NH):
                    finish_unit(h, 0, pv0[h], out0)
                dma_out(0, out0)

            # ---- scope 2: qb1-3, software-pipelined S/exp/PV
            with tc.tile_pool(name="ps_qb", bufs=2, space="PSUM") as ps_qb:
                for qb in range(1, NQB):
                    out_t = out_tiles()
                    for h in range(NH):
                        pv_t = ps_pv.tile([128, 512], f32, tag="pv", name="pvq")
                        pending = []  # (p_tile, kt_pair)
                        for pr in range(8):
                            kt0 = 2 * pr
                            s_t = ps_qb.tile([128, 1024], f32, tag="s2", name="s2")
                            s_mm(h, kt0, qb, s_t[:, 0:512])
                            s_mm(h, kt0 + 1, qb, s_t[:, 512:1024])
                            if len(pending) >= 2:
                                p_prev, kp = pending.pop(0)
                                for half in range(2):
                                    for qsb in range(4):
                                        pv_mm(
                                            pv_t, h, 2 * kp + half,
                                            p_prev[:, half * 512 : (half + 1) * 512],
                                            qsb,
                                            first=(kp == 0 and half == 0 and qsb == 0),
                                            last=False,
                                        )
                            p_t = p_pool.tile([128, 1024], bf16, tag="p1", name="p1")
                            exp_to(p_t, s_t, "act" if pr % 2 == 0 else "dve")
                            pending.append((p_t, pr))
                        for p_prev, kp in pending:
                            for half in range(2):
                                for qsb in range(4):
                                    pv_mm(
                                        pv_t, h, 2 * kp + half,
                                        p_prev[:, half * 512 : (half + 1) * 512],
                                        qsb,
                                        first=False,
                                        last=(kp == 7 and half == 1 and qsb == 3),
                                    )
                        finish_unit(h, qb, pv_t, out_t)
                    dma_out(qb, out_t)

    nc.compile()
    return nc


def kernel(query, keys, Wq, Wk, Wv):
    from concourse.bass_utils import run_bass_kernel_spmd

    if "nc" not in _cache:
        _cache["nc"] = _build()
    nc = _cache["nc"]

    query = np.asarray(query, dtype=np.float32)
    keys = np.asarray(keys, dtype=np.float32)
    Wq = np.asarray(Wq, dtype=np.float32)
    Wk = np.asarray(Wk, dtype=np.float32)
    Wv = np.asarray(Wv, dtype=np.float32)
    B = query.shape[0]
    assert query.shape == (4, L, D) and keys.shape == (4, L, D)
    assert Wq.shape == (D, D) and Wk.shape == (D, D) and Wv.shape == (D, D)

    in_maps = []
    for c in range(8):
        b, hg = c // 2, c % 2
        sl = slice(hg * DHG, (hg + 1) * DHG)
        # permute q/wq feature columns so this core's residual channels are
        # columns 0:256 on device (Q = q @ Wq^T invariant to column perm)
        perm = np.r_[hg * DHG : (hg + 1) * DHG, (1 - hg) * DHG : (2 - hg) * DHG]
        # permute wq/wk ROWS to (i, h, k') order so the projection emits the
        # DoubleRow head-shuffled layout with contiguous weight slices:
        # device row i*128 + h*32 + k'  <-  channel h*64 + i*32 + k'
        rperm = np.array(
            [h * 64 + i * 32 + kk for i in range(2) for h in range(NH) for kk in range(32)]
        )
        in_maps.append(
            {
                "q": np.ascontiguousarray(query[b][:, perm]),
                "k": np.ascontiguousarray(keys[b]),
                "wq": np.ascontiguousarray(Wq[sl][:, perm][rperm]),
                "wk": np.ascontiguousarray(Wk[sl][rperm]),
                "wv": np.ascontiguousarray(Wv[sl]),
            }
        )
    res = run_bass_kernel_spmd(nc, in_maps, list(range(8)), **_cache.get("run_kwargs", {}))
    _cache["last_result"] = res
    out = np.empty((B, L, D), np.float32)
    for c in range(8):
        b, hg = c // 2, c % 2
        out[b][:, hg * DHG : (hg + 1) * DHG] = res.results[c]["o"]
    return out


# revision 6
# speedup vs baseline: 1.3085x; 1.3085x over previous
# Multi-head attention (B=4, L=2048, D=512, H=8, dh=64) on 8 trn2 cores.
# Sharding: core c -> batch b = c//2, head-group hg = c%2 (4 heads, 256 out
# channels). Host permutes q/wq feature columns so the residual slice is
# always columns 0:256, and permutes wq/wk rows to (i, h, k') order so the
# projection emits the DoubleRow head-shuffled layout directly.
#
# Per-core plan:
#   - batched DMA loads of q/k/w natural fp32; Pool casts to bf16; DMA XBAR
#     transposes (one per row-tile) -> qT/kT/wT (no PE transposes).
#   - QT/KT projections (bf16): PSUM partition h*32+k' for sub-row i,
#     copy-cast fp8e4m3 on Act -> QT8/KT8 [128, 2*2048].
#   - S^T = K_h Q_h^T as fp8 DoubleRow matmuls (contraction 64 = 32x2).
#   - exp: Act exact exp->bf16; DVE Schraudolph int16 bitcast bf16 (softmax
#     ratio cancels the common-mode scale).
#   - PV natural: lhsT = P^T chunk [128k,128q], rhs = V+ones [128,65] ->
#     PSUM [128q, 65] accumulated over 16 key blocks; col 64 = denominator.
#     Normalize+residual fused on DVE; batched store per q-block.
import sys

import numpy as np

sys.path.insert(0, "/opt/trn_rl_repo")

L = 2048
D = 512
NH = 4          # heads per core
DH = 64
DHG = NH * DH   # 256 output channels per core
NLT = L // 128  # 16 row tiles
NCI = D // 128  # 4 feature chunks
QB = 512        # q block
NQB = L // QB   # 4
INV_SCALE = 1.0 / float(np.sqrt(D))
# Schraudolph exp in bf16-int space: int16 = round(x*128/ln2 + 16255.4)
SCH_C1 = float(128.0 / np.log(2.0) * INV_SCALE)
SCH_C2 = float(127.0 * 128.0 - 0.6)

_cache = {}


def _build():
    import concourse.bacc as bacc
    import concourse.mybir as mybir
    import concourse.tile as tile

    f32 = mybir.dt.float32
    bf16 = mybir.dt.bfloat16
    i16 = mybir.dt.int16
    fp8 = mybir.dt.float8e4
    EXP = mybir.ActivationFunctionType.Exp
    CPY = mybir.ActivationFunctionType.Copy
    MUL = mybir.AluOpType.mult
    ADD = mybir.AluOpType.add
    DR = mybir.MatmulPerfMode.DoubleRow

    nc = bacc.Bacc("TRN2", target_bir_lowering=False, debug=False, num_devices=8)
    q_d = nc.dram_tensor("q", [L, D], f32, kind="ExternalInput").ap()
    k_d = nc.dram_tensor("k", [L, D], f32, kind="ExternalInput").ap()
    wq_d = nc.dram_tensor("wq", [DHG, D], f32, kind="ExternalInput").ap()
    wk_d = nc.dram_tensor("wk", [DHG, D], f32, kind="ExternalInput").ap()
    wv_d = nc.dram_tensor("wv", [DHG, D], f32, kind="ExternalInput").ap()
    o_d = nc.dram_tensor("o", [L, DHG], f32, kind="ExternalOutput").ap()

    with tile.TileContext(nc) as tc:
        with (
            tc.tile_pool(name="static", bufs=1) as st_pool,
            tc.tile_pool(name="bfc", bufs=4) as bf_pool,
            tc.tile_pool(name="ppool", bufs=5) as p_pool,
            tc.tile_pool(name="outsb", bufs=2) as out_pool,
            tc.tile_pool(name="recip", bufs=5) as r_pool,
            tc.tile_pool(name="ps_pv", bufs=4, space="PSUM") as ps_pv,
        ):
            # ---- static tiles
            qnat = st_pool.tile([128, NLT * 512], f32, name="qnat")
            knat = st_pool.tile([128, NLT * 512], f32, name="knat")
            wnat = [st_pool.tile([128, 2 * 512], f32, name=f"wn{x}") for x in range(3)]
            # transposed, chunk-contiguous: cols [c*L, (c+1)*L) = feature chunk c
            qT = st_pool.tile([128, NCI * L], bf16, name="qT")
            kT = st_pool.tile([128, NCI * L], bf16, name="kT")
            wT = [st_pool.tile([128, NCI * DHG], bf16, name=f"wT{x}") for x in range(3)]
            QT8 = st_pool.tile([128, 2 * L], fp8, name="QT8")
            KT8 = st_pool.tile([128, 2 * L], fp8, name="KT8")
            V_all = [
                st_pool.tile([128, NH * (DH + 1)], bf16, name=f"V{kt}")
                for kt in range(NLT)
            ]

            # ---- batched loads
            nc.sync.dma_start(
                out=qnat.rearrange("p (lt d) -> p lt d", lt=NLT),
                in_=q_d.rearrange("(lt p) d -> p lt d", p=128),
            )
            nc.sync.dma_start(
                out=knat.rearrange("p (lt d) -> p lt d", lt=NLT),
                in_=k_d.rearrange("(lt p) d -> p lt d", p=128),
            )
            for x, w_d in enumerate((wq_d, wk_d, wv_d)):
                nc.sync.dma_start(
                    out=wnat[x].rearrange("p (oc d) -> p oc d", oc=2),
                    in_=w_d.rearrange("(oc p) d -> p oc d", p=128),
                )

            # ---- casts (Pool) + XBAR transposes (SP), one per row-tile
            for x in range(3):
                for oc in range(2):
                    wb = bf_pool.tile([128, 512], bf16, tag="wb", name="wb")
                    nc.gpsimd.tensor_copy(wb, wnat[x][:, oc * 512 : (oc + 1) * 512])
                    nc.sync.dma_start_transpose(
                        wT[x].rearrange("p (c o) -> p c o", c=NCI)[
                            :, :, oc * 128 : (oc + 1) * 128
                        ],
                        wb,
                    )
            for kt in range(NLT):
                view = V_all[kt].rearrange("p (h x) -> p h x", h=NH)[:, :, DH : DH + 1]
                nc.gpsimd.memset(view, 1.0)
            for lt in range(NLT):
                qb_ = bf_pool.tile([128, 512], bf16, tag="qb", name="qb")
                nc.gpsimd.tensor_copy(qb_, qnat[:, lt * 512 : (lt + 1) * 512])
                nc.sync.dma_start_transpose(
                    qT.rearrange("p (c l) -> p c l", c=NCI)[
                        :, :, lt * 128 : (lt + 1) * 128
                    ],
                    qb_,
                )
            for lt in range(NLT):
                kb_ = bf_pool.tile([128, 512], bf16, tag="kb", name="kb")
                nc.gpsimd.tensor_copy(kb_, knat[:, lt * 512 : (lt + 1) * 512])
                nc.sync.dma_start_transpose(
                    kT.rearrange("p (c l) -> p c l", c=NCI)[
                        :, :, lt * 128 : (lt + 1) * 128
                    ],
                    kb_,
                )

            def proj_qk(x, tT, dst8, lb, i):
                ps = ps_k.tile([128, 512], f32, tag="s1", name="pjps")
                for ci in range(NCI):
                    nc.tensor.matmul(
                        ps,
                        lhsT=wT[x][:, ci * DHG + i * 128 : ci * DHG + (i + 1) * 128],
                        rhs=tT[:, ci * L + lb * QB : ci * L + (lb + 1) * QB],
                        start=(ci == 0),
                        stop=(ci == NCI - 1),
                    )
                dst = dst8[:, i * L + lb * QB : i * L + (lb + 1) * QB]
                nc.scalar.activation(dst, ps, CPY)

            def v_proj(kt):
                ps = ps_k.tile([128, 512], f32, tag="s1", name="vps")
                for ci in range(NCI):
                    nc.tensor.matmul(
                        ps[:, 0:DHG],
                        lhsT=kT[:, ci * L + kt * 128 : ci * L + (kt + 1) * 128],
                        rhs=wT[2][:, ci * DHG : (ci + 1) * DHG],
                        start=(ci == 0),
                        stop=(ci == NCI - 1),
                    )
                dst = V_all[kt].rearrange("p (h x) -> p h x", h=NH)[:, :, 0:DH]
                src = ps[:, 0:DHG].rearrange("p (h x) -> p h x", h=NH)
                nc.scalar.activation(dst, src, CPY)

            def dr_views(t8, h):
                return t8[32 * h : 32 * (h + 1), :].rearrange("p (i l) -> p i l", i=2)

            def s_mm(h, kt, qb, out_ap):
                # S^T [128 keys, 512 q] fp8 DoubleRow
                nc.tensor.matmul(
                    out_ap,
                    lhsT=dr_views(KT8, h)[:, :, kt * 128 : (kt + 1) * 128],
                    rhs=dr_views(QT8, h)[:, :, qb * QB : (qb + 1) * QB],
                    start=True,
                    stop=True,
                    perf_mode=DR,
                    tile_position=(32 * h, 0),
                )

            def exp_to(p_dst, s_src, eng):
                if eng == "act":
                    nc.scalar.activation(p_dst, s_src, EXP, scale=INV_SCALE)
                else:
                    nc.vector.tensor_scalar(
                        p_dst.bitcast(i16), s_src, SCH_C1, SCH_C2, MUL, ADD
                    )

            def pv_mm(pv_t, h, kt, p_ap, qsb, first, last):
                nc.tensor.matmul(
                    pv_t[:, qsb * (DH + 1) : (qsb + 1) * (DH + 1)],
                    lhsT=p_ap[:, qsb * 128 : (qsb + 1) * 128],
                    rhs=V_all[kt][:, h * (DH + 1) : (h + 1) * (DH + 1)],
                    start=first,
                    stop=last,
                    skip_group_check=True,
                )

            def finish_unit(h, qb, pv_t, out_t):
                rc = r_pool.tile([128, 4], f32, tag="rc", name="rc")
                sums = pv_t[:, 0 : 4 * (DH + 1)].rearrange(
                    "p (q x) -> p q x", x=DH + 1
                )[:, :, DH]
                nc.vector.reciprocal(rc, sums)
                for qsb in range(4):
                    nc.vector.scalar_tensor_tensor(
                        out=out_t[:, qsb * DHG + h * DH : qsb * DHG + (h + 1) * DH],
                        in0=pv_t[:, qsb * (DH + 1) : qsb * (DH + 1) + DH],
                        scalar=rc[:, qsb : qsb + 1],
                        in1=qnat[:, (qb * 4 + qsb) * 512 + h * DH : (qb * 4 + qsb) * 512 + (h + 1) * DH],
                        op0=MUL,
                        op1=ADD,
                    )

            def dma_out(qb, out_t):
                nc.sync.dma_start(
                    out=o_d[qb * QB : (qb + 1) * QB, :].rearrange(
                        "(qsb p) d -> p qsb d", p=128
                    ),
                    in_=out_t.rearrange("p (qsb d) -> p qsb d", qsb=4),
                )

            # ---- scope 1: projections + K production + qb0 attention
            with tc.tile_pool(name="ps_k", bufs=4, space="PSUM") as ps_k:
                for i in range(2):
                    for lb in range(NQB):
                        proj_qk(0, qT, QT8, lb, i)
                pv0 = [
                    ps_pv.tile([128, 512], f32, tag="pv", name=f"pv0{h}")
                    for h in range(NH)
                ]
                out0 = out_pool.tile([128, 4 * DHG], f32, tag="ot", name="ot0")
                expi = 0
                for lb in range(NQB):
                    for i in range(2):
                        proj_qk(1, kT, KT8, lb, i)
                    for kt in range(lb * 4, lb * 4 + 4):
                        v_proj(kt)
                    for kt in range(lb * 4, lb * 4 + 4):
                        for h in range(NH):
                            s_t = ps_k.tile([128, 512], f32, tag="s1", name="s0")
                            s_mm(h, kt, 0, s_t)
                            p_t = p_pool.tile([128, 512], bf16, tag="p0", name="p0")
                            exp_to(p_t, s_t, "act" if expi % 2 == 0 else "dve")
                            expi += 1
                            for qsb in range(4):
                                pv_mm(
                                    pv0[h], h, kt, p_t, qsb,
                                    first=(kt == 0 and qsb == 0),
                                    last=(kt == NLT - 1 and qsb == 3),
                                )
                for h in range(NH):
                    finish_unit(h, 0, pv0[h], out0)
                dma_out(0, out0)

            # ---- scope 2: qb1-3, software-pipelined S/exp/PV
            with tc.tile_pool(name="ps_qb", bufs=2, space="PSUM") as ps_qb:
                for qb in range(1, NQB):
                    out_t = out_pool.tile([128, 4 * DHG], f32, tag="ot", name="otq")
                    for h in range(NH):
                        pv_t = ps_pv.tile([128, 512], f32, tag="pv", name="pvq")
                        pending = []  # (p_tile, pair_idx)
                        for pr in range(8):
                            kt0 = 2 * pr
                            s_t = ps_qb.tile([128, 1024], f32, tag="s2", name="s2")
                            s_mm(h, kt0, qb, s_t[:, 0:512])
                            s_mm(h, kt0 + 1, qb, s_t[:, 512:1024])
                            if len(pending) >= 2:
                                p_prev, kp = pending.pop(0)
                                for half in range(2):
                                    for qsb in range(4):
                                        pv_mm(
                                            pv_t, h, 2 * kp + half,
                                            p_prev[:, half * 512 : (half + 1) * 512],
                                            qsb,
                                            first=(kp == 0 and half == 0 and qsb == 0),
                                            last=False,
                                        )
                            p_t = p_pool.tile([128, 1024], bf16, tag="p1", name="p1")
                            exp_to(p_t, s_t, "act" if pr % 2 == 0 else "dve")
                            pending.append((p_t, pr))
                        for p_prev, kp in pending:
                            for half in range(2):
                                for qsb in range(4):
                                    pv_mm(
                                        pv_t, h, 2 * kp + half,
                                        p_prev[:, half * 512 : (half + 1) * 512],
                                        qsb,
                                        first=False,
                                        last=(kp == 7 and half == 1 and qsb == 3),
                                    )
                        finish_unit(h, qb, pv_t, out_t)
                    dma_out(qb, out_t)

    nc.compile()
    return nc


def kernel(query, keys, Wq, Wk, Wv):
    from concourse.bass_utils import run_bass_kernel_spmd

    if "nc" not in _cache:
        _cache["nc"] = _build()
    nc = _cache["nc"]

    query = np.asarray(query, dtype=np.float32)
    keys = np.asarray(keys, dtype=np.float32)
    Wq = np.asarray(Wq, dtype=np.float32)
    Wk = np.asarray(Wk, dtype=np.float32)
    Wv = np.asarray(Wv, dtype=np.float32)
    B = query.shape[0]
    assert query.shape == (4, L, D) and keys.shape == (4, L, D)
    assert Wq.shape == (D, D) and Wk.shape == (D, D) and Wv.shape == (D, D)

    in_maps = []
    for c in range(8):
        b, hg = c // 2, c % 2
        sl = slice(hg * DHG, (hg + 1) * DHG)
        # permute q/wq feature columns so this core's residual channels are
        # columns 0:256 on device (Q = q @ Wq^T invariant to column perm)
        perm = np.r_[hg * DHG : (hg + 1) * DHG, (1 - hg) * DHG : (2 - hg) * DHG]
        # permute wq/wk ROWS to (i, h, k') order so the projection emits the
        # DoubleRow head-shuffled layout with contiguous weight slices:
        # device row i*128 + h*32 + k'  <-  channel h*64 + i*32 + k'
        rperm = np.array(
            [h * 64 + i * 32 + kk for i in range(2) for h in range(NH) for kk in range(32)]
        )
        in_maps.append(
            {
                "q": np.ascontiguousarray(query[b][:, perm]),
                "k": np.ascontiguousarray(keys[b]),
                "wq": np.ascontiguousarray(Wq[sl][:, perm][rperm]),
                "wk": np.ascontiguousarray(Wk[sl][rperm]),
                "wv": np.ascontiguousarray(Wv[sl]),
            }
        )
    res = run_bass_kernel_spmd(nc, in_maps, list(range(8)), **_cache.get("run_kwargs", {}))
    _cache["last_result"] = res
    out = np.empty((B, L, D), np.float32)
    for c in range(8):
        b, hg = c // 2, c % 2
        out[b][:, hg * DHG : (hg + 1) * DHG] = res.results[c]["o"]
    return out


# revision 7
# speedup vs baseline: 1.4505x; 1.1085x over previous
# Multi-head attention (B=4, L=2048, D=512, H=8, dh=64) on 8 trn2 cores.
# Sharding: core c -> batch b = c//2, head-group hg = c%2 (4 heads, 256 out
# channels). Host marshalling: q/k/w cast to bf16; q/wq feature columns
# permuted so the residual slice is columns 0:256; wq/wk rows permuted to
# (i, h, k') order so the projection emits the DoubleRow head-shuffled
# layout directly.
#
# Per-core plan:
#   - bf16 loads; DMA XBAR transposes -> qT/kT/wT (no PE transposes, no
#     casts).
#   - QT/KT projections (bf16): PSUM partition h*32+k' for sub-row i,
#     copy-cast fp8e4m3 on Act -> QT8/KT8 [128, 2*2048].
#   - S^T = K_h Q_h^T as fp8 DoubleRow matmuls (contraction 64 = 32x2).
#   - exp: Act exact exp->bf16; DVE Schraudolph int16 bitcast bf16 (softmax
#     ratio cancels the common-mode scale). 19:13 Act:DVE split.
#   - PV natural: lhsT = P^T chunk [128k,128q], rhs = V+ones [128,65] ->
#     PSUM [128q, 65] accumulated over 16 key blocks; col 64 = denominator.
#     Normalize+residual fused on DVE; batched store per q-block.
import sys

import numpy as np

sys.path.insert(0, "/opt/trn_rl_repo")

L = 2048
D = 512
NH = 4          # heads per core
DH = 64
DHG = NH * DH   # 256 output channels per core
NLT = L // 128  # 16 row tiles
NCI = D // 128  # 4 feature chunks
QB = 512        # q block
NQB = L // QB   # 4
INV_SCALE = 1.0 / float(np.sqrt(D))
# Schraudolph exp in bf16-int space: int16 = round(x*128/ln2 + 16255.4)
SCH_C1 = float(128.0 / np.log(2.0) * INV_SCALE)
SCH_C2 = float(127.0 * 128.0 - 0.6)

_cache = {}


def _build():
    import concourse.bacc as bacc
    import concourse.mybir as mybir
    import concourse.tile as tile

    f32 = mybir.dt.float32
    bf16 = mybir.dt.bfloat16
    i16 = mybir.dt.int16
    fp8 = mybir.dt.float8e4
    EXP = mybir.ActivationFunctionType.Exp
    CPY = mybir.ActivationFunctionType.Copy
    MUL = mybir.AluOpType.mult
    ADD = mybir.AluOpType.add
    DR = mybir.MatmulPerfMode.DoubleRow

    nc = bacc.Bacc("TRN2", target_bir_lowering=False, debug=False, num_devices=8)
    q_d = nc.dram_tensor("q", [L, D], bf16, kind="ExternalInput").ap()
    k_d = nc.dram_tensor("k", [L, D], bf16, kind="ExternalInput").ap()
    wq_d = nc.dram_tensor("wq", [DHG, D], bf16, kind="ExternalInput").ap()
    wk_d = nc.dram_tensor("wk", [DHG, D], bf16, kind="ExternalInput").ap()
    wv_d = nc.dram_tensor("wv", [DHG, D], bf16, kind="ExternalInput").ap()
    o_d = nc.dram_tensor("o", [L, DHG], f32, kind="ExternalOutput").ap()

    with tile.TileContext(nc) as tc:
        with (
            tc.tile_pool(name="static", bufs=1) as st_pool,
            tc.tile_pool(name="ppool", bufs=5) as p_pool,
            tc.tile_pool(name="outsb", bufs=2) as out_pool,
            tc.tile_pool(name="recip", bufs=5) as r_pool,
            tc.tile_pool(name="ps_pv", bufs=4, space="PSUM") as ps_pv,
        ):
            # ---- static tiles (all bf16)
            qnat = st_pool.tile([128, NLT * 512], bf16, name="qnat")
            knat = st_pool.tile([128, NLT * 512], bf16, name="knat")
            wnat = [st_pool.tile([128, 2 * 512], bf16, name=f"wn{x}") for x in range(3)]
            # transposed, chunk-contiguous: cols [c*L, (c+1)*L) = feature chunk c
            qT = st_pool.tile([128, NCI * L], bf16, name="qT")
            kT = st_pool.tile([128, NCI * L], bf16, name="kT")
            wT = [st_pool.tile([128, NCI * DHG], bf16, name=f"wT{x}") for x in range(3)]
            QT8 = st_pool.tile([128, 2 * L], fp8, name="QT8")
            KT8 = st_pool.tile([128, 2 * L], fp8, name="KT8")
            V_all = [
                st_pool.tile([128, NH * (DH + 1)], bf16, name=f"V{kt}")
                for kt in range(NLT)
            ]

            # ---- loads: w, then interleaved q/k quarters
            for x, w_d in enumerate((wq_d, wk_d, wv_d)):
                nc.sync.dma_start(
                    out=wnat[x].rearrange("p (oc d) -> p oc d", oc=2),
                    in_=w_d.rearrange("(oc p) d -> p oc d", p=128),
                )
            for lb in range(NQB):
                for nat, t_d in ((qnat, q_d), (knat, k_d)):
                    nc.sync.dma_start(
                        out=nat.rearrange("p (lt d) -> p lt d", lt=NLT)[
                            :, lb * 4 : (lb + 1) * 4, :
                        ],
                        in_=t_d.rearrange("(lt p) d -> p lt d", p=128)[
                            :, lb * 4 : (lb + 1) * 4, :
                        ],
                    )

            # ---- XBAR transposes (SP), one per row-tile
            for x in range(3):
                for oc in range(2):
                    nc.sync.dma_start_transpose(
                        wT[x].rearrange("p (c o) -> p c o", c=NCI)[
                            :, :, oc * 128 : (oc + 1) * 128
                        ],
                        wnat[x][:, oc * 512 : (oc + 1) * 512],
                    )
            for kt in range(NLT):
                view = V_all[kt].rearrange("p (h x) -> p h x", h=NH)[:, :, DH : DH + 1]
                nc.gpsimd.memset(view, 1.0)
            for lb in range(NQB):
                for lt in range(lb * 4, lb * 4 + 4):
                    nc.sync.dma_start_transpose(
                        qT.rearrange("p (c l) -> p c l", c=NCI)[
                            :, :, lt * 128 : (lt + 1) * 128
                        ],
                        qnat[:, lt * 512 : (lt + 1) * 512],
                    )
                for lt in range(lb * 4, lb * 4 + 4):
                    nc.sync.dma_start_transpose(
                        kT.rearrange("p (c l) -> p c l", c=NCI)[
                            :, :, lt * 128 : (lt + 1) * 128
                        ],
                        knat[:, lt * 512 : (lt + 1) * 512],
                    )

            def proj_qk(x, tT, dst8, lb, i):
                ps = ps_k.tile([128, 512], f32, tag="s1", name="pjps")
                for ci in range(NCI):
                    nc.tensor.matmul(
                        ps,
                        lhsT=wT[x][:, ci * DHG + i * 128 : ci * DHG + (i + 1) * 128],
                        rhs=tT[:, ci * L + lb * QB : ci * L + (lb + 1) * QB],
                        start=(ci == 0),
                        stop=(ci == NCI - 1),
                    )
                dst = dst8[:, i * L + lb * QB : i * L + (lb + 1) * QB]
                nc.scalar.activation(dst, ps, CPY)

            def v_proj(kt, eng):
                ps = ps_k.tile([128, 512], f32, tag="s1", name="vps")
                for ci in range(NCI):
                    nc.tensor.matmul(
                        ps[:, 0:DHG],
                        lhsT=kT[:, ci * L + kt * 128 : ci * L + (kt + 1) * 128],
                        rhs=wT[2][:, ci * DHG : (ci + 1) * DHG],
                        start=(ci == 0),
                        stop=(ci == NCI - 1),
                    )
                dst = V_all[kt].rearrange("p (h x) -> p h x", h=NH)[:, :, 0:DH]
                src = ps[:, 0:DHG].rearrange("p (h x) -> p h x", h=NH)
                if eng == "act":
                    nc.scalar.activation(dst, src, CPY)
                else:
                    nc.vector.tensor_copy(dst, src)

            def dr_views(t8, h):
                return t8[32 * h : 32 * (h + 1), :].rearrange("p (i l) -> p i l", i=2)

            def s_mm(h, kt, qb, out_ap):
                nc.tensor.matmul(
                    out_ap,
                    lhsT=dr_views(KT8, h)[:, :, kt * 128 : (kt + 1) * 128],
                    rhs=dr_views(QT8, h)[:, :, qb * QB : (qb + 1) * QB],
                    start=True,
                    stop=True,
                    perf_mode=DR,
                    tile_position=(32 * h, 0),
                )

            def exp_to(p_dst, s_src, eng):
                if eng == "act":
                    nc.scalar.activation(p_dst, s_src, EXP, scale=INV_SCALE)
                else:
                    nc.vector.tensor_scalar(
                        p_dst.bitcast(i16), s_src, SCH_C1, SCH_C2, MUL, ADD
                    )

            def pv_mm(pv_t, h, kt, p_ap, qsb, first, last):
                nc.tensor.matmul(
                    pv_t[:, qsb * (DH + 1) : (qsb + 1) * (DH + 1)],
                    lhsT=p_ap[:, qsb * 128 : (qsb + 1) * 128],
                    rhs=V_all[kt][:, h * (DH + 1) : (h + 1) * (DH + 1)],
                    start=first,
                    stop=last,
                    skip_group_check=True,
                )

            def finish_unit(h, qb, pv_t, out_t):
                rc = r_pool.tile([128, 4], f32, tag="rc", name="rc")
                sums = pv_t[:, 0 : 4 * (DH + 1)].rearrange(
                    "p (q x) -> p q x", x=DH + 1
                )[:, :, DH]
                nc.vector.reciprocal(rc, sums)
                for qsb in range(4):
                    nc.vector.scalar_tensor_tensor(
                        out=out_t[:, qsb * DHG + h * DH : qsb * DHG + (h + 1) * DH],
                        in0=pv_t[:, qsb * (DH + 1) : qsb * (DH + 1) + DH],
                        scalar=rc[:, qsb : qsb + 1],
                        in1=qnat[:, (qb * 4 + qsb) * 512 + h * DH : (qb * 4 + qsb) * 512 + (h + 1) * DH],
                        op0=MUL,
                        op1=ADD,
                    )

            def dma_out(qb, out_t):
                nc.sync.dma_start(
                    out=o_d[qb * QB : (qb + 1) * QB, :].rearrange(
                        "(qsb p) d -> p qsb d", p=128
                    ),
                    in_=out_t.rearrange("p (qsb d) -> p qsb d", qsb=4),
                )

            # ---- scope 1: projections + K production + qb0 attention
            with tc.tile_pool(name="ps_k", bufs=4, space="PSUM") as ps_k:
                for i in range(2):
                    for lb in range(NQB):
                        proj_qk(0, qT, QT8, lb, i)
                pv0 = [
                    ps_pv.tile([128, 512], f32, tag="pv", name=f"pv0{h}")
                    for h in range(NH)
                ]
                out0 = out_pool.tile([128, 4 * DHG], f32, tag="ot", name="ot0")
                expi = 0
                for lb in range(NQB):
                    for i in range(2):
                        proj_qk(1, kT, KT8, lb, i)
                    for j, kt in enumerate(range(lb * 4, lb * 4 + 4)):
                        v_proj(kt, "act" if j % 2 == 0 else "dve")
                    for kt in range(lb * 4, lb * 4 + 4):
                        for h in range(NH):
                            s_t = ps_k.tile([128, 512], f32, tag="s1", name="s0")
                            s_mm(h, kt, 0, s_t)
                            p_t = p_pool.tile([128, 512], bf16, tag="p0", name="p0")
                            exp_to(p_t, s_t, "act" if expi % 2 == 0 else "dve")
                            expi += 1
                            for qsb in range(4):
                                pv_mm(
                                    pv0[h], h, kt, p_t, qsb,
                                    first=(kt == 0 and qsb == 0),
                                    last=(kt == NLT - 1 and qsb == 3),
                                )
                for h in range(NH):
                    finish_unit(h, 0, pv0[h], out0)
                dma_out(0, out0)

            # ---- scope 2: qb1-3, flattened software pipeline across (qb, h)
            with tc.tile_pool(name="ps_qb", bufs=2, space="PSUM") as ps_qb:
                steps = [
                    (qb, h, pr)
                    for qb in range(1, NQB)
                    for h in range(NH)
                    for pr in range(8)
                ]
                # exp engine pattern: 19 act / 13 dve per 32 steps, spread evenly
                eng_of = ["act" if (i * 19) % 32 < 19 else "dve" for i in range(32)]
                pending = []  # (p_tile, qb, h, pr, pv_t, out_t)
                unit_state = {}
                out_ts = {}

                def drain_one():
                    p_prev, dqb, dh, dpr, dpv, dout = pending.pop(0)
                    for half in range(2):
                        for qsb in range(4):
                            pv_mm(
                                dpv, dh, 2 * dpr + half,
                                p_prev[:, half * 512 : (half + 1) * 512],
                                qsb,
                                first=(dpr == 0 and half == 0 and qsb == 0),
                                last=(dpr == 7 and half == 1 and qsb == 3),
                            )
                    if dpr == 7:
                        finish_unit(dh, dqb, dpv, dout)
                        if dh == NH - 1:
                            dma_out(dqb, dout)

                for si, (qb, h, pr) in enumerate(steps):
                    if pr == 0:
                        if h == 0:
                            out_ts[qb] = out_pool.tile(
                                [128, 4 * DHG], f32, tag="ot", name="otq"
                            )
                        unit_state[(qb, h)] = ps_pv.tile(
                            [128, 512], f32, tag="pv", name="pvq"
                        )
                    pv_t = unit_state[(qb, h)]
                    s_t = ps_qb.tile([128, 1024], f32, tag="s2", name="s2")
                    s_mm(h, 2 * pr, qb, s_t[:, 0:512])
                    s_mm(h, 2 * pr + 1, qb, s_t[:, 512:1024])
                    if len(pending) >= 2:
                        drain_one()
                    p_t = p_pool.tile([128, 1024], bf16, tag="p1", name="p1")
                    exp_to(p_t, s_t, eng_of[si % 32])
                    pending.append((p_t, qb, h, pr, pv_t, out_ts[qb]))
                while pending:
                    drain_one()

    nc.compile()
    return nc


def kernel(query, keys, Wq, Wk, Wv):
    import ml_dtypes

    from concourse.bass_utils import run_bass_kernel_spmd

    if "nc" not in _cache:
        _cache["nc"] = _build()
    nc = _cache["nc"]

    query = np.asarray(query, dtype=np.float32)
    keys = np.asarray(keys, dtype=np.float32)
    Wq = np.asarray(Wq, dtype=np.float32)
    Wk = np.asarray(Wk, dtype=np.float32)
    Wv = np.asarray(Wv, dtype=np.float32)
    B = query.shape[0]
    assert query.shape == (4, L, D) and keys.shape == (4, L, D)
    assert Wq.shape == (D, D) and Wk.shape == (D, D) and Wv.shape == (D, D)

    bf = ml_dtypes.bfloat16
    in_maps = []
    for c in range(8):
        b, hg = c // 2, c % 2
        sl = slice(hg * DHG, (hg + 1) * DHG)
        # permute q/wq feature columns so this core's residual channels are
        # columns 0:256 on device (Q = q @ Wq^T invariant to column perm)
        perm = np.r_[hg * DHG : (hg + 1) * DHG, (1 - hg) * DHG : (2 - hg) * DHG]
        # permute wq/wk ROWS to (i, h, k') order so the projection emits the
        # DoubleRow head-shuffled layout with contiguous weight slices:
        # device row i*128 + h*32 + k'  <-  channel h*64 + i*32 + k'
        rperm = np.array(
            [h * 64 + i * 32 + kk for i in range(2) for h in range(NH) for kk in range(32)]
        )
        in_maps.append(
            {
                "q": np.ascontiguousarray(query[b][:, perm].astype(bf)),
                "k": np.ascontiguousarray(keys[b].astype(bf)),
                "wq": np.ascontiguousarray(Wq[sl][:, perm][rperm].astype(bf)),
                "wk": np.ascontiguousarray(Wk[sl][rperm].astype(bf)),
                "wv": np.ascontiguousarray(Wv[sl].astype(bf)),
            }
        )
    res = run_bass_kernel_spmd(nc, in_maps, list(range(8)), **_cache.get("run_kwargs", {}))
    _cache["last_result"] = res
    out = np.empty((B, L, D), np.float32)
    for c in range(8):
        b, hg = c // 2, c % 2
        out[b][:, hg * DHG : (hg + 1) * DHG] = res.results[c]["o"]
    return out


# revision 8
# speedup vs baseline: 1.6710x; 1.1520x over previous
# Multi-head attention (B=4, L=2048, D=512, H=8, dh=64) on 8 trn2 cores.
# Sharding: core c -> batch b = c//2, head-group hg = c%2 (4 heads, 256 out
# channels). Host marshalling: q/k/w cast to bf16; q/wq feature columns
# permuted so the residual slice is columns 0:256; wq/wk rows permuted to
# (i, h, k') order so the projection emits the DoubleRow head-shuffled
# layout directly.
#
# Per-core plan:
#   - bf16 loads; DMA XBAR transposes -> qT/kT/wT (no PE transposes, no
#     casts).
#   - QT/KT projections (bf16): PSUM partition h*32+k' for sub-row i,
#     copy-cast fp8e4m3 on Act -> QT8/KT8 [128, 2*2048].
#   - S^T = K_h Q_h^T as fp8 DoubleRow matmuls (contraction 64 = 32x2).
#   - exp: Act exact exp->bf16; DVE Schraudolph int16 bitcast bf16 (softmax
#     ratio cancels the common-mode scale). 19:13 Act:DVE split.
#   - PV natural: lhsT = P^T chunk [128k,128q], rhs = V+ones [128,65] ->
#     PSUM [128q, 65] accumulated over 16 key blocks; col 64 = denominator.
#     Normalize+residual fused on DVE; batched store per q-block.
import sys

import numpy as np

sys.path.insert(0, "/opt/trn_rl_repo")

L = 2048
D = 512
NH = 4          # heads per core
DH = 64
DHG = NH * DH   # 256 output channels per core
NLT = L // 128  # 16 row tiles
NCI = D // 128  # 4 feature chunks
QB = 512        # q block
NQB = L // QB   # 4
INV_SCALE = 1.0 / float(np.sqrt(D))
# Schraudolph exp in bf16-int space: int16 = round(x*128/ln2 + 16255.4)
SCH_C1 = float(128.0 / np.log(2.0) * INV_SCALE)
SCH_C2 = float(127.0 * 128.0 - 0.6)

_cache = {}


def _build():
    import concourse.bacc as bacc
    import concourse.mybir as mybir
    import concourse.tile as tile

    f32 = mybir.dt.float32
    bf16 = mybir.dt.bfloat16
    i16 = mybir.dt.int16
    fp8 = mybir.dt.float8e4
    EXP = mybir.ActivationFunctionType.Exp
    CPY = mybir.ActivationFunctionType.Copy
    MUL = mybir.AluOpType.mult
    ADD = mybir.AluOpType.add
    DR = mybir.MatmulPerfMode.DoubleRow

    nc = bacc.Bacc("TRN2", target_bir_lowering=False, debug=False, num_devices=8)
    q_d = nc.dram_tensor("q", [L, D], bf16, kind="ExternalInput").ap()
    k_d = nc.dram_tensor("k", [L, D], bf16, kind="ExternalInput").ap()
    wq_d = nc.dram_tensor("wq", [DHG, D], bf16, kind="ExternalInput").ap()
    wk_d = nc.dram_tensor("wk", [DHG, D], bf16, kind="ExternalInput").ap()
    wv_d = nc.dram_tensor("wv", [DHG, D], bf16, kind="ExternalInput").ap()
    o_d = nc.dram_tensor("o", [L, DHG], f32, kind="ExternalOutput").ap()

    with tile.TileContext(nc) as tc:
        with (
            tc.tile_pool(name="static", bufs=1) as st_pool,
            tc.tile_pool(name="ppool", bufs=5) as p_pool,
            tc.tile_pool(name="outsb", bufs=2) as out_pool,
            tc.tile_pool(name="recip", bufs=5) as r_pool,
            tc.tile_pool(name="ps_pv", bufs=4, space="PSUM") as ps_pv,
        ):
            # ---- static tiles (all bf16)
            qres = st_pool.tile([128, NLT * DHG], bf16, name="qres")
            # transposed, chunk-contiguous: cols [c*L, (c+1)*L) = feature chunk c
            qT = st_pool.tile([128, NCI * L], bf16, name="qT")
            kT = st_pool.tile([128, NCI * L], bf16, name="kT")
            wT = [st_pool.tile([128, NCI * DHG], bf16, name=f"wT{x}") for x in range(3)]
            QT8 = st_pool.tile([128, 2 * L], fp8, name="QT8")
            KT8 = st_pool.tile([128, 2 * L], fp8, name="KT8")
            V_all = [
                st_pool.tile([128, NH * (DH + 1)], bf16, name=f"V{kt}")
                for kt in range(NLT)
            ]

            # ---- DRAM-direct XBAR transposes (bf16) + qres slice load
            for x, w_d in enumerate((wq_d, wk_d, wv_d)):
                nc.sync.dma_start_transpose(
                    wT[x].rearrange("p (c o) -> p c o", c=NCI), w_d
                )
            for kt in range(NLT):
                view = V_all[kt].rearrange("p (h x) -> p h x", h=NH)[:, :, DH : DH + 1]
                nc.gpsimd.memset(view, 1.0)
            for lb in range(NQB):
                nc.sync.dma_start_transpose(
                    qT.rearrange("p (c l) -> p c l", c=NCI)[
                        :, :, lb * QB : (lb + 1) * QB
                    ],
                    q_d[lb * QB : (lb + 1) * QB, :],
                )
                nc.sync.dma_start_transpose(
                    kT.rearrange("p (c l) -> p c l", c=NCI)[
                        :, :, lb * QB : (lb + 1) * QB
                    ],
                    k_d[lb * QB : (lb + 1) * QB, :],
                )
            nc.sync.dma_start(
                out=qres.rearrange("p (lt d) -> p lt d", lt=NLT),
                in_=q_d.rearrange("(lt p) d -> p lt d", p=128)[:, :, 0:DHG],
            )

            def proj_qk(x, tT, dst8, lb, i):
                ps = ps_k.tile([128, 512], f32, tag="s1", name="pjps")
                for ci in range(NCI):
                    nc.tensor.matmul(
                        ps,
                        lhsT=wT[x][:, ci * DHG + i * 128 : ci * DHG + (i + 1) * 128],
                        rhs=tT[:, ci * L + lb * QB : ci * L + (lb + 1) * QB],
                        start=(ci == 0),
                        stop=(ci == NCI - 1),
                    )
                dst = dst8[:, i * L + lb * QB : i * L + (lb + 1) * QB]
                nc.scalar.activation(dst, ps, CPY)

            def v_proj(kt, eng):
                ps = ps_k.tile([128, 512], f32, tag="s1", name="vps")
                for ci in range(NCI):
                    nc.tensor.matmul(
                        ps[:, 0:DHG],
                        lhsT=kT[:, ci * L + kt * 128 : ci * L + (kt + 1) * 128],
                        rhs=wT[2][:, ci * DHG : (ci + 1) * DHG],
                        start=(ci == 0),
                        stop=(ci == NCI - 1),
                    )
                dst = V_all[kt].rearrange("p (h x) -> p h x", h=NH)[:, :, 0:DH]
                src = ps[:, 0:DHG].rearrange("p (h x) -> p h x", h=NH)
                if eng == "act":
                    nc.scalar.activation(dst, src, CPY)
                else:
                    nc.vector.tensor_copy(dst, src)

            def dr_views(t8, h):
                return t8[32 * h : 32 * (h + 1), :].rearrange("p (i l) -> p i l", i=2)

            def s_mm(h, kt, qb, out_ap):
                nc.tensor.matmul(
                    out_ap,
                    lhsT=dr_views(KT8, h)[:, :, kt * 128 : (kt + 1) * 128],
                    rhs=dr_views(QT8, h)[:, :, qb * QB : (qb + 1) * QB],
                    start=True,
                    stop=True,
                    perf_mode=DR,
                    tile_position=(32 * h, 0),
                )

            def exp_to(p_dst, s_src, eng):
                if eng == "act":
                    nc.scalar.activation(p_dst, s_src, EXP, scale=INV_SCALE)
                else:
                    nc.vector.tensor_scalar(
                        p_dst.bitcast(i16), s_src, SCH_C1, SCH_C2, MUL, ADD
                    )

            def pv_mm(pv_t, h, kt, p_ap, qsb, first, last):
                nc.tensor.matmul(
                    pv_t[:, qsb * (DH + 1) : (qsb + 1) * (DH + 1)],
                    lhsT=p_ap[:, qsb * 128 : (qsb + 1) * 128],
                    rhs=V_all[kt][:, h * (DH + 1) : (h + 1) * (DH + 1)],
                    start=first,
                    stop=last,
                    skip_group_check=True,
                )

            def finish_unit(h, qb, pv_t, out_t):
                rc = r_pool.tile([128, 4], f32, tag="rc", name="rc")
                sums = pv_t[:, 0 : 4 * (DH + 1)].rearrange(
                    "p (q x) -> p q x", x=DH + 1
                )[:, :, DH]
                nc.vector.reciprocal(rc, sums)
                for qsb in range(4):
                    nc.vector.scalar_tensor_tensor(
                        out=out_t[:, qsb * DHG + h * DH : qsb * DHG + (h + 1) * DH],
                        in0=pv_t[:, qsb * (DH + 1) : qsb * (DH + 1) + DH],
                        scalar=rc[:, qsb : qsb + 1],
                        in1=qres[:, (qb * 4 + qsb) * DHG + h * DH : (qb * 4 + qsb) * DHG + (h + 1) * DH],
                        op0=MUL,
                        op1=ADD,
                    )

            def dma_out(qb, out_t):
                nc.sync.dma_start(
                    out=o_d[qb * QB : (qb + 1) * QB, :].rearrange(
                        "(qsb p) d -> p qsb d", p=128
                    ),
                    in_=out_t.rearrange("p (qsb d) -> p qsb d", qsb=4),
                )

            # ---- scope 1: projections + K production + qb0 attention
            with tc.tile_pool(name="ps_k", bufs=4, space="PSUM") as ps_k:
                for i in range(2):
                    for lb in range(NQB):
                        proj_qk(0, qT, QT8, lb, i)
                pv0 = [
                    ps_pv.tile([128, 512], f32, tag="pv", name=f"pv0{h}")
                    for h in range(NH)
                ]
                out0 = out_pool.tile([128, 4 * DHG], f32, tag="ot", name="ot0")
                expi = 0
                for lb in range(NQB):
                    for i in range(2):
                        proj_qk(1, kT, KT8, lb, i)
                    for kt in range(lb * 4, lb * 4 + 4):
                        v_proj(kt, "dve")
                    for kt in range(lb * 4, lb * 4 + 4):
                        for h in range(NH):
                            s_t = ps_k.tile([128, 512], f32, tag="s1", name="s0")
                            s_mm(h, kt, 0, s_t)
                            p_t = p_pool.tile([128, 512], bf16, tag="p0", name="p0")
                            exp_to(p_t, s_t, "act" if (expi * 9) % 16 < 9 else "dve")
                            expi += 1
                            for qsb in range(4):
                                pv_mm(
                                    pv0[h], h, kt, p_t, qsb,
                                    first=(kt == 0 and qsb == 0),
                                    last=(kt == NLT - 1 and qsb == 3),
                                )
                for h in range(NH):
                    finish_unit(h, 0, pv0[h], out0)
                dma_out(0, out0)

            # ---- scope 2: qb1-3, flattened software pipeline across (qb, h)
            with tc.tile_pool(name="ps_qb", bufs=2, space="PSUM") as ps_qb:
                steps = [
                    (qb, h, pr)
                    for qb in range(1, NQB)
                    for h in range(NH)
                    for pr in range(8)
                ]
                # exp engine pattern: 19 act / 13 dve per 32 steps, spread evenly
                eng_of = ["act" if (i * 19) % 32 < 19 else "dve" for i in range(32)]
                pending = []  # (p_tile, qb, h, pr, pv_t, out_t)
                unit_state = {}
                out_ts = {}

                def drain_one():
                    p_prev, dqb, dh, dpr, dpv, dout = pending.pop(0)
                    for half in range(2):
                        for qsb in range(4):
                            pv_mm(
                                dpv, dh, 2 * dpr + half,
                                p_prev[:, half * 512 : (half + 1) * 512],
                                qsb,
                                first=(dpr == 0 and half == 0 and qsb == 0),
                                last=(dpr == 7 and half == 1 and qsb == 3),
                            )
                    if dpr == 7:
                        finish_unit(dh, dqb, dpv, dout)
                        if dh == NH - 1:
                            dma_out(dqb, dout)

                for si, (qb, h, pr) in enumerate(steps):
                    if pr == 0:
                        if h == 0:
                            out_ts[qb] = out_pool.tile(
                                [128, 4 * DHG], f32, tag="ot", name="otq"
                            )
                        unit_state[(qb, h)] = ps_pv.tile(
                            [128, 512], f32, tag="pv", name="pvq"
                        )
                    pv_t = unit_state[(qb, h)]
                    s_t = ps_qb.tile([128, 1024], f32, tag="s2", name="s2")
                    s_mm(h, 2 * pr, qb, s_t[:, 0:512])
                    s_mm(h, 2 * pr + 1, qb, s_t[:, 512:1024])
                    if len(pending) >= 2:
                        drain_one()
                    p_t = p_pool.tile([128, 1024], bf16, tag="p1", name="p1")
                    exp_to(p_t, s_t, eng_of[si % 32])
                    pending.append((p_t, qb, h, pr, pv_t, out_ts[qb]))
                while pending:
                    drain_one()

    nc.compile()
    return nc


def kernel(query, keys, Wq, Wk, Wv):
    import ml_dtypes

    from concourse.bass_utils import run_bass_kernel_spmd

    if "nc" not in _cache:
        _cache["nc"] = _build()
    nc = _cache["nc"]

    query = np.asarray(query, dtype=np.float32)
    keys = np.asarray(keys, dtype=np.float32)
    Wq = np.asarray(Wq, dtype=np.float32)
    Wk = np.asarray(Wk, dtype=np.float32)
    Wv = np.asarray(Wv, dtype=np.float32)
    B = query.shape[0]
    assert query.shape == (4, L, D) and keys.shape == (4, L, D)
    assert Wq.shape == (D, D) and Wk.shape == (D, D) and Wv.shape == (D, D)

    bf = ml_dtypes.bfloat16
    in_maps = []
    for c in range(8):
        b, hg = c // 2, c % 2
        sl = slice(hg * DHG, (hg + 1) * DHG)
        # permute q/wq feature columns so this core's residual channels are
        # columns 0:256 on device (Q = q @ Wq^T invariant to column perm)
        perm = np.r_[hg * DHG : (hg + 1) * DHG, (1 - hg) * DHG : (2 - hg) * DHG]
        # permute wq/wk ROWS to (i, h, k') order so the projection emits the
        # DoubleRow head-shuffled layout with contiguous weight slices:
        # device row i*128 + h*32 + k'  <-  channel h*64 + i*32 + k'
        rperm = np.array(
            [h * 64 + i * 32 + kk for i in range(2) for h in range(NH) for kk in range(32)]
        )
        in_maps.append(
            {
                "q": np.ascontiguousarray(query[b][:, perm].astype(bf)),
                "k": np.ascontiguousarray(keys[b].astype(bf)),
                "wq": np.ascontiguousarray(Wq[sl][:, perm][rperm].astype(bf)),
                "wk": np.ascontiguousarray(Wk[sl][rperm].astype(bf)),
                "wv": np.ascontiguousarray(Wv[sl].astype(bf)),
            }
        )
    res = run_bass_kernel_spmd(nc, in_maps, list(range(8)), **_cache.get("run_kwargs", {}))
    _cache["last_result"] = res
    out = np.empty((B, L, D), np.float32)
    for c in range(8):
        b, hg = c // 2, c % 2
        out[b][:, hg * DHG : (hg + 1) * DHG] = res.results[c]["o"]
    return out


# revision 9
# speedup vs baseline: 1.9098x; 1.1429x over previous
# Multi-head attention (B=4, L=2048, D=512, H=8, dh=64) on 8 trn2 cores.
# Sharding: core c -> batch b = c//2, head-group hg = c%2 (4 heads, 256 out
# channels). Host marshalling: q/k/w cast to bf16; q/wq feature columns
# permuted so the residual slice is columns 0:256; wq/wk rows permuted to
# (i, h, k') order so the projection emits the DoubleRow head-shuffled
# layout directly.
#
# Per-core plan:
#   - bf16 loads; DMA XBAR transposes -> qT/kT/wT (no PE transposes, no
#     casts).
#   - QT/KT projections (bf16): PSUM partition h*32+k' for sub-row i,
#     copy-cast fp8e4m3 on Act -> QT8/KT8 [128, 2*2048].
#   - S^T = K_h Q_h^T as fp8 DoubleRow matmuls (contraction 64 = 32x2).
#   - exp: Act exact exp->bf16; DVE Schraudolph int16 bitcast bf16 (softmax
#     ratio cancels the common-mode scale). 19:13 Act:DVE split.
#   - PV natural: lhsT = P^T chunk [128k,128q], rhs = V+ones [128,65] ->
#     PSUM [128q, 65] accumulated over 16 key blocks; col 64 = denominator.
#     Normalize+residual fused on DVE; batched store per q-block.
import sys

import numpy as np

sys.path.insert(0, "/opt/trn_rl_repo")

L = 2048
D = 512
NH = 4          # heads per core
DH = 64
DHG = NH * DH   # 256 output channels per core
NLT = L // 128  # 16 row tiles
NCI = D // 128  # 4 feature chunks
QB = 512        # q block
NQB = L // QB   # 4
INV_SCALE = 1.0 / float(np.sqrt(D))
# Schraudolph exp in bf16-int space: int16 = round(x*128/ln2 + 16255.4)
SCH_C1 = float(128.0 / np.log(2.0) * INV_SCALE)
SCH_C2 = float(127.0 * 128.0 - 0.6)

_cache = {}


def _build():
    import concourse.bacc as bacc
    import concourse.mybir as mybir
    import concourse.tile as tile

    f32 = mybir.dt.float32
    bf16 = mybir.dt.bfloat16
    i16 = mybir.dt.int16
    fp8 = mybir.dt.float8e4
    EXP = mybir.ActivationFunctionType.Exp
    CPY = mybir.ActivationFunctionType.Copy
    MUL = mybir.AluOpType.mult
    ADD = mybir.AluOpType.add
    DR = mybir.MatmulPerfMode.DoubleRow

    nc = bacc.Bacc("TRN2", target_bir_lowering=False, debug=False, num_devices=8)
    q_d = nc.dram_tensor("q", [L, D], bf16, kind="ExternalInput").ap()
    k_d = nc.dram_tensor("k", [L, D], bf16, kind="ExternalInput").ap()
    wq_d = nc.dram_tensor("wq", [DHG, D], bf16, kind="ExternalInput").ap()
    wk_d = nc.dram_tensor("wk", [DHG, D], bf16, kind="ExternalInput").ap()
    wv_d = nc.dram_tensor("wv", [DHG, D], bf16, kind="ExternalInput").ap()
    o_d = nc.dram_tensor("o", [L, DHG], f32, kind="ExternalOutput").ap()

    with tile.TileContext(nc) as tc:
        with (
            tc.tile_pool(name="static", bufs=1) as st_pool,
            tc.tile_pool(name="ppool", bufs=6) as p_pool,
            tc.tile_pool(name="outsb", bufs=2) as out_pool,
            tc.tile_pool(name="recip", bufs=5) as r_pool,
        ):
            # ---- static tiles (all bf16)
            qres = st_pool.tile([128, NLT * DHG], bf16, name="qres")
            # transposed, chunk-contiguous: cols [c*L, (c+1)*L) = feature chunk c
            qT = st_pool.tile([128, NCI * L], bf16, name="qT")
            kT = st_pool.tile([128, NCI * L], bf16, name="kT")
            wT = [st_pool.tile([128, NCI * DHG], bf16, name=f"wT{x}") for x in range(3)]
            QT8 = st_pool.tile([128, 2 * L], fp8, name="QT8")
            KT8 = st_pool.tile([128, 2 * L], fp8, name="KT8")
            V_all = [
                st_pool.tile([128, NH * (DH + 1)], bf16, name=f"V{kt}")
                for kt in range(NLT)
            ]

            # ---- DRAM-direct XBAR transposes (bf16) + qres slice load
            for x, w_d in enumerate((wq_d, wk_d, wv_d)):
                nc.sync.dma_start_transpose(
                    wT[x].rearrange("p (c o) -> p c o", c=NCI), w_d
                )
            for kt in range(NLT):
                view = V_all[kt].rearrange("p (h x) -> p h x", h=NH)[:, :, DH : DH + 1]
                nc.gpsimd.memset(view, 1.0)
            for lb in range(NQB):
                nc.sync.dma_start_transpose(
                    qT.rearrange("p (c l) -> p c l", c=NCI)[
                        :, :, lb * QB : (lb + 1) * QB
                    ],
                    q_d[lb * QB : (lb + 1) * QB, :],
                )
                nc.sync.dma_start_transpose(
                    kT.rearrange("p (c l) -> p c l", c=NCI)[
                        :, :, lb * QB : (lb + 1) * QB
                    ],
                    k_d[lb * QB : (lb + 1) * QB, :],
                )
            nc.sync.dma_start(
                out=qres.rearrange("p (lt d) -> p lt d", lt=NLT),
                in_=q_d.rearrange("(lt p) d -> p lt d", p=128)[:, :, 0:DHG],
            )

            def proj_qk(x, tT, dst8, lb, i, copy_eng="act"):
                ps = ps_k.tile([128, 512], f32, tag="s1", name="pjps")
                for ci in range(NCI):
                    nc.tensor.matmul(
                        ps,
                        lhsT=wT[x][:, ci * DHG + i * 128 : ci * DHG + (i + 1) * 128],
                        rhs=tT[:, ci * L + lb * QB : ci * L + (lb + 1) * QB],
                        start=(ci == 0),
                        stop=(ci == NCI - 1),
                    )
                dst = dst8[:, i * L + lb * QB : i * L + (lb + 1) * QB]
                if copy_eng == "act":
                    nc.scalar.activation(dst, ps, CPY)
                else:
                    nc.vector.tensor_copy(dst, ps)

            def v_proj(kt, eng):
                ps = ps_k.tile([128, 512], f32, tag="s1", name="vps")
                for ci in range(NCI):
                    nc.tensor.matmul(
                        ps[:, 0:DHG],
                        lhsT=kT[:, ci * L + kt * 128 : ci * L + (kt + 1) * 128],
                        rhs=wT[2][:, ci * DHG : (ci + 1) * DHG],
                        start=(ci == 0),
                        stop=(ci == NCI - 1),
                    )
                dst = V_all[kt].rearrange("p (h x) -> p h x", h=NH)[:, :, 0:DH]
                src = ps[:, 0:DHG].rearrange("p (h x) -> p h x", h=NH)
                if eng == "act":
                    nc.scalar.activation(dst, src, CPY)
                else:
                    nc.vector.tensor_copy(dst, src)

            def dr_views(t8, h):
                return t8[32 * h : 32 * (h + 1), :].rearrange("p (i l) -> p i l", i=2)

            def s_mm(h, kt, qb, out_ap):
                nc.tensor.matmul(
                    out_ap,
                    lhsT=dr_views(KT8, h)[:, :, kt * 128 : (kt + 1) * 128],
                    rhs=dr_views(QT8, h)[:, :, qb * QB : (qb + 1) * QB],
                    start=True,
                    stop=True,
                    perf_mode=DR,
                    tile_position=(32 * h, 0),
                )

            def exp_to(p_dst, s_src, eng):
                if eng == "act":
                    nc.scalar.activation(p_dst, s_src, EXP, scale=INV_SCALE)
                else:
                    nc.vector.tensor_scalar(
                        p_dst.bitcast(i16), s_src, SCH_C1, SCH_C2, MUL, ADD
                    )

            def pv_mm(pv_t, h, kt, p_ap, qsb, first, last):
                nc.tensor.matmul(
                    pv_t[:, qsb * (DH + 1) : (qsb + 1) * (DH + 1)],
                    lhsT=p_ap[:, qsb * 128 : (qsb + 1) * 128],
                    rhs=V_all[kt][:, h * (DH + 1) : (h + 1) * (DH + 1)],
                    start=first,
                    stop=last,
                    skip_group_check=True,
                )

            def finish_unit(h, qb, pv_t, out_t):
                rc = r_pool.tile([128, 4], f32, tag="rc", name="rc")
                sums = pv_t[:, 0 : 4 * (DH + 1)].rearrange(
                    "p (q x) -> p q x", x=DH + 1
                )[:, :, DH]
                nc.vector.reciprocal(rc, sums)
                for qsb in range(4):
                    nc.vector.scalar_tensor_tensor(
                        out=out_t[:, qsb * DHG + h * DH : qsb * DHG + (h + 1) * DH],
                        in0=pv_t[:, qsb * (DH + 1) : qsb * (DH + 1) + DH],
                        scalar=rc[:, qsb : qsb + 1],
                        in1=qres[:, (qb * 4 + qsb) * DHG + h * DH : (qb * 4 + qsb) * DHG + (h + 1) * DH],
                        op0=MUL,
                        op1=ADD,
                    )

            def dma_out(qb, out_t):
                nc.sync.dma_start(
                    out=o_d[qb * QB : (qb + 1) * QB, :].rearrange(
                        "(qsb p) d -> p qsb d", p=128
                    ),
                    in_=out_t.rearrange("p (qsb d) -> p qsb d", qsb=4),
                )

            # ---- scope 1: projections + K production + qb0 attention
            with (
                tc.tile_pool(name="ps_k", bufs=4, space="PSUM") as ps_k,
                tc.tile_pool(name="ps_pv4", bufs=4, space="PSUM") as ps_pv,
            ):
                for i in range(2):
                    for lb in range(NQB):
                        proj_qk(0, qT, QT8, lb, i)
                pv0 = [
                    ps_pv.tile([128, 512], f32, tag="pv", name=f"pv0{h}")
                    for h in range(NH)
                ]
                out0 = out_pool.tile([128, 4 * DHG], f32, tag="ot", name="ot0")
                expi = 0
                for lb in range(NQB):
                    for i in range(2):
                        proj_qk(1, kT, KT8, lb, i, "dve")
                    for kt in range(lb * 4, lb * 4 + 4):
                        v_proj(kt, "act")
                    for kt in range(lb * 4, lb * 4 + 4):
                        for h in range(NH):
                            s_t = ps_k.tile([128, 512], f32, tag="s1", name="s0")
                            s_mm(h, kt, 0, s_t)
                            p_t = p_pool.tile([128, 512], bf16, tag="p0", name="p0")
                            exp_to(p_t, s_t, "act" if (expi * 9) % 16 < 9 else "dve")
                            expi += 1
                            for qsb in range(4):
                                pv_mm(
                                    pv0[h], h, kt, p_t, qsb,
                                    first=(kt == 0 and qsb == 0),
                                    last=(kt == NLT - 1 and qsb == 3),
                                )
                for h in range(NH):
                    finish_unit(h, 0, pv0[h], out0)
                dma_out(0, out0)

            # ---- scope 2: qb1-3, flattened software pipeline across (qb, h)
            with (
                tc.tile_pool(name="ps_qb", bufs=3, space="PSUM") as ps_qb,
                tc.tile_pool(name="ps_pv2", bufs=2, space="PSUM") as ps_pv,
            ):
                steps = [
                    (qb, h, pr)
                    for qb in range(1, NQB)
                    for h in range(NH)
                    for pr in range(8)
                ]
                # exp engine pattern: 19 act / 13 dve per 32 steps, spread evenly
                eng_of = ["act" if (i * 19) % 32 < 19 else "dve" for i in range(32)]
                pending = []  # (p_tile, qb, h, pr, pv_t, out_t)
                unit_state = {}
                out_ts = {}

                def drain_one():
                    p_prev, dqb, dh, dpr, dpv, dout = pending.pop(0)
                    for half in range(2):
                        for qsb in range(4):
                            pv_mm(
                                dpv, dh, 2 * dpr + half,
                                p_prev[:, half * 512 : (half + 1) * 512],
                                qsb,
                                first=(dpr == 0 and half == 0 and qsb == 0),
                                last=(dpr == 7 and half == 1 and qsb == 3),
                            )
                    if dpr == 7:
                        finish_unit(dh, dqb, dpv, dout)
                        if dh == NH - 1:
                            dma_out(dqb, dout)

                for si, (qb, h, pr) in enumerate(steps):
                    if pr == 0:
                        if h == 0:
                            out_ts[qb] = out_pool.tile(
                                [128, 4 * DHG], f32, tag="ot", name="otq"
                            )
                        unit_state[(qb, h)] = ps_pv.tile(
                            [128, 512], f32, tag="pv", name="pvq"
                        )
                    pv_t = unit_state[(qb, h)]
                    s_t = ps_qb.tile([128, 1024], f32, tag="s2", name="s2")
                    s_mm(h, 2 * pr, qb, s_t[:, 0:512])
                    s_mm(h, 2 * pr + 1, qb, s_t[:, 512:1024])
                    if len(pending) >= 3:
                        drain_one()
                    p_t = p_pool.tile([128, 1024], bf16, tag="p1", name="p1")
                    exp_to(p_t, s_t, eng_of[si % 32])
                    pending.append((p_t, qb, h, pr, pv_t, out_ts[qb]))
                while pending:
                    drain_one()

    nc.compile()
    return nc


def kernel(query, keys, Wq, Wk, Wv):
    import ml_dtypes

    from concourse.bass_utils import run_bass_kernel_spmd

    if "nc" not in _cache:
        _cache["nc"] = _build()
    nc = _cache["nc"]

    query = np.asarray(query, dtype=np.float32)
    keys = np.asarray(keys, dtype=np.float32)
    Wq = np.asarray(Wq, dtype=np.float32)
    Wk = np.asarray(Wk, dtype=np.float32)
    Wv = np.asarray(Wv, dtype=np.float32)
    B = query.shape[0]
    assert query.shape == (4, L, D) and keys.shape == (4, L, D)
    assert Wq.shape == (D, D) and Wk.shape == (D, D) and Wv.shape == (D, D)

    bf = ml_dtypes.bfloat16
    in_maps = []
    for c in range(8):
        b, hg = c // 2, c % 2
        sl = slice(hg * DHG, (hg + 1) * DHG)
        # permute q/wq feature columns so this core's residual channels are
        # columns 0:256 on device (Q = q @ Wq^T invariant to column perm)
        perm = np.r_[hg * DHG : (hg + 1) * DHG, (1 - hg) * DHG : (2 - hg) * DHG]
        # permute wq/wk ROWS to (i, h, k') order so the projection emits the
        # DoubleRow head-shuffled layout with contiguous weight slices:
        # device row i*128 + h*32 + k'  <-  channel h*64 + i*32 + k'
        rperm = np.array(
            [h * 64 + i * 32 + kk for i in range(2) for h in range(NH) for kk in range(32)]
        )
        in_maps.append(
            {
                "q": np.ascontiguousarray(query[b][:, perm].astype(bf)),
                "k": np.ascontiguousarray(keys[b].astype(bf)),
                "wq": np.ascontiguousarray(Wq[sl][:, perm][rperm].astype(bf)),
                "wk": np.ascontiguousarray(Wk[sl][rperm].astype(bf)),
                "wv": np.ascontiguousarray(Wv[sl].astype(bf)),
            }
        )
    res = run_bass_kernel_spmd(nc, in_maps, list(range(8)), **_cache.get("run_kwargs", {}))
    _cache["last_result"] = res
    out = np.empty((B, L, D), np.float32)
    for c in range(8):
        b, hg = c // 2, c % 2
        out[b][:, hg * DHG : (hg + 1) * DHG] = res.results[c]["o"]
    return out
